# revision 8
# baseline (speedup 1.0000x reference)
"""Soft-VQ codebook kernel for Trainium2, 8 NeuronCores, vocab-parallel.

Reference computation (B=128, K=8, d_model=768, d=512, V=49408):
  kw    = keywords @ W_proj + b_proj            # [1024, 512]
  cos   = (kw/|kw|) @ (W_emb/|W_emb|).T         # [1024, V]
  prob  = softmax(cos, axis=-1)
  kwout = prob @ W_emb                          # [1024, 512]
returns (kwout, prob, cos) reshaped to [128, 8, ...].

Sharding: vocab axis of W_emb split 8 ways (6176 rows/core, ~6.3MB bf16
resident in SBUF in both [v,d] and [d,v] layouts). Every core computes all
1024 rows against its vocab shard. Softmax needs no max subtraction
(|cos| <= 1), so the only cross-core exchange is an AllReduce-add of the
per-row exp-sums (one tiny AllReduce per 128-row block) plus one AllReduce
of the [1024, 512] output partial sums.
"""

import os

import numpy as np
import ml_dtypes

import concourse.bass as bass
import concourse.tile as tile
from concourse import bacc, mybir
from concourse.bass_utils import run_bass_kernel_spmd
from concourse.masks import make_identity

F32 = mybir.dt.float32
BF16 = mybir.dt.bfloat16
AF = mybir.ActivationFunctionType

NCORES = 8
B, KWD, DM, D, V = 128, 8, 768, 512, 49408
NR = B * KWD            # 1024 rows
VSH = V // NCORES       # 6176 vocab rows per core
RG = [list(range(NCORES))]

# v-tiles of 512 along the shard vocab axis (12 full + one 32-wide tail)
VT512 = [(j * 512, min(512, VSH - j * 512)) for j in range((VSH + 511) // 512)]
# v-chunks of 128 (48 full + one 32-wide tail)
VT128 = [(k * 128, min(128, VSH - k * 128)) for k in range((VSH + 127) // 128)]
NBLK = NR // 128        # 8 row blocks of 128


def build_nc():
    nc = bacc.Bacc("TRN2", target_bir_lowering=False, debug=False,
                   num_devices=NCORES)

    kwT_d = nc.dram_tensor("kwT", [DM, NR], F32, kind="ExternalInput")
    wproj_d = nc.dram_tensor("wproj", [DM, D], F32, kind="ExternalInput")
    bproj_d = nc.dram_tensor("bproj", [1, D], F32, kind="ExternalInput")
    wembT_d = nc.dram_tensor("wembT", [D, VSH], BF16, kind="ExternalInput")
    wemb_d = nc.dram_tensor("wemb", [VSH, D], BF16, kind="ExternalInput")

    scores_d = nc.dram_tensor("scores", [NR, VSH], F32, kind="ExternalOutput")
    probs_d = nc.dram_tensor("probs", [NR, VSH], F32, kind="ExternalOutput")
    kwout_d = nc.dram_tensor("kwout", [NR, D], F32, kind="ExternalOutput")

    with tile.TileContext(nc) as tc:
        build_tile_kernel(nc, tc, kwT_d, wproj_d, bproj_d, wembT_d, wemb_d,
                          scores_d, probs_d, kwout_d)
    if not nc.is_finalized():
        nc.finalize()
    return nc


def build_tile_kernel(nc, tc, kwT_d, wproj_d, bproj_d, wembT_d, wemb_d,
                      scores_d, probs_d, kwout_d):
    from contextlib import ExitStack

    outer = ExitStack()
    persist = outer.enter_context(tc.tile_pool(name="persist", bufs=1))

    # ---- persistent SBUF ----
    # transposed shard [d, v]: 4 partition-tiles; normalized in place below
    embT = persist.tile([128, 4, VSH], BF16)
    nc.sync.dma_start(out=embT,
                      in_=wembT_d.ap().rearrange("(i p) v -> p i v", p=128))
    # natural shard [v, d]: 49 chunks of <=128 rows (raw W_emb, out-matmul rhs)
    emb = persist.tile([128, len(VT128), D], BF16)
    nfull = (VSH // 128) * 128
    nc.sync.dma_start(out=emb[:, :VSH // 128, :],
                      in_=wemb_d.ap()[:nfull].rearrange("(k p) d -> p k d", p=128))
    tail = VSH - nfull
    if tail:
        nc.sync.dma_start(out=emb[:tail, VSH // 128, :], in_=wemb_d.ap()[nfull:])

    kw_nT = persist.tile([128, 4, NR], BF16)       # normalized kw, transposed
    scales = persist.tile([128, NBLK], F32)        # 1/gsum per row block
    ident = persist.tile([128, 128], BF16)
    make_identity(nc, ident)
    ones_bf = persist.tile([128, 1], BF16)
    nc.vector.memset(ones_bf, 1.0)
    ones_row = persist.tile([1, 128], F32)
    nc.vector.memset(ones_row, 1.0)

    # ---- setup: normalize embT in place;  compute kw_nT ----
    with ExitStack() as setup:
        sp = setup.enter_context(tc.tile_pool(name="setup", bufs=2))
        kwp = setup.enter_context(tc.tile_pool(name="kwsetup", bufs=1))
        ps_n = setup.enter_context(tc.tile_pool(name="psn", bufs=2, space="PSUM"))
        ps_b = setup.enter_context(tc.tile_pool(name="psb", bufs=2, space="PSUM"))
        ps_kw = setup.enter_context(tc.tile_pool(name="pskw", bufs=2, space="PSUM"))
        ps_t = setup.enter_context(tc.tile_pool(name="pst", bufs=2, space="PSUM"))

        # vocab norms: norm2[v] = sum_d W[v,d]^2 via ones-matmul over 4 d-tiles
        for v0, w in VT512:
            n2 = ps_n.tile([1, 512], F32)
            for i in range(4):
                sq = sp.tile([128, 512], BF16, tag="sq")
                nc.vector.tensor_mul(sq[:, :w], embT[:, i, v0:v0 + w],
                                     embT[:, i, v0:v0 + w])
                nc.tensor.matmul(n2[:, :w], lhsT=ones_bf, rhs=sq[:, :w],
                                 start=(i == 0), stop=(i == 3))
            nrm = sp.tile([1, 512], F32, tag="nrm")
            nc.scalar.sqrt(nrm[:, :w], n2[:, :w])
            inv = sp.tile([1, 512], F32, tag="inv")
            nc.vector.reciprocal(inv[:, :w], nrm[:, :w])
            bc = ps_b.tile([128, 512], F32)
            nc.tensor.matmul(bc[:, :w], lhsT=ones_row, rhs=inv[:, :w],
                             start=True, stop=True)
            for i in range(4):
                nc.vector.tensor_mul(embT[:, i, v0:v0 + w],
                                     embT[:, i, v0:v0 + w], bc[:, :w])

        # kw = keywords @ W_proj + b;  kw_n = kw/|kw|;  kw_nT = transpose
        kwT_sb = kwp.tile([128, 6, NR], F32)
        nc.sync.dma_start(out=kwT_sb,
                          in_=kwT_d.ap().rearrange("(i p) r -> p i r", p=128))
        wproj_sb = kwp.tile([128, 6, D], F32)
        nc.sync.dma_start(out=wproj_sb,
                          in_=wproj_d.ap().rearrange("(i p) d -> p i d", p=128))
        bproj_sb = kwp.tile([1, D], F32)
        nc.sync.dma_start(out=bproj_sb, in_=bproj_d.ap())

        for rc in range(NBLK):
            kw_ps = ps_kw.tile([128, D], F32)
            for i in range(6):
                nc.tensor.matmul(kw_ps, lhsT=kwT_sb[:, i, rc * 128:(rc + 1) * 128],
                                 rhs=wproj_sb[:, i, :], start=(i == 0), stop=False)
            # bias add: += ones[128] (x) b_proj  (K=1 matmul)
            nc.tensor.matmul(kw_ps, lhsT=ones_row, rhs=bproj_sb,
                             start=False, stop=True)
            sqk = sp.tile([128, D], F32, tag="sqk")
            n2k = sp.tile([128, 1], F32, tag="n2k")
            nc.scalar.activation(sqk, kw_ps, AF.Square, accum_out=n2k)
            nrk = sp.tile([128, 1], F32, tag="nrk")
            nc.scalar.sqrt(nrk, n2k)
            ivk = sp.tile([128, 1], F32, tag="ivk")
            nc.vector.reciprocal(ivk, nrk)
            kwn = sp.tile([128, D], BF16, tag="kwn")
            nc.vector.tensor_scalar_mul(kwn, kw_ps, ivk)
            for i in range(4):
                tp = ps_t.tile([128, 128], BF16)
                nc.tensor.transpose(tp, kwn[:, i * 128:(i + 1) * 128], ident)
                nc.scalar.copy(kw_nT[:, i, rc * 128:(rc + 1) * 128], tp)

    # ---- main pools ----
    with ExitStack() as main:
        ep = main.enter_context(tc.tile_pool(name="efwd", bufs=2))
        scp = main.enter_context(tc.tile_pool(name="scstage", bufs=4))
        prp = main.enter_context(tc.tile_pool(name="prstage", bufs=4))
        lsp = main.enter_context(tc.tile_pool(name="lsums", bufs=2))
        stp = main.enter_context(tc.tile_pool(name="stats", bufs=4))
        etp = main.enter_context(tc.tile_pool(name="et", bufs=3))
        accsb = main.enter_context(tc.tile_pool(name="accsb", bufs=4))
        ps_s = main.enter_context(tc.tile_pool(name="pss", bufs=2, space="PSUM"))
        ps_st = main.enter_context(tc.tile_pool(name="psst", bufs=2, space="PSUM"))
        ps_acc = main.enter_context(tc.tile_pool(name="psacc", bufs=1, space="PSUM"))
        dram = main.enter_context(tc.tile_pool(name="dram", bufs=4, space="DRAM"))
        dram_acc = main.enter_context(tc.tile_pool(name="dramacc", bufs=2,
                                                   space="DRAM"))

        # ---- S phases: scores, exp, row sums, AllReduce, probs ----
        for b in range(NBLK):
            r0 = b * 128
            E = ep.tile([128, VSH], BF16)
            lsums = lsp.tile([128, len(VT512)], F32)
            for j, (v0, w) in enumerate(VT512):
                ps = ps_s.tile([128, 512], F32)
                for i in range(4):
                    nc.tensor.matmul(ps[:, :w],
                                     lhsT=kw_nT[:, i, r0:r0 + 128],
                                     rhs=embT[:, i, v0:v0 + w],
                                     start=(i == 0), stop=(i == 3))
                sc = scp.tile([128, 512], F32)
                nc.vector.tensor_copy(sc[:, :w], ps[:, :w])
                nc.sync.dma_start(out=scores_d.ap()[r0:r0 + 128, v0:v0 + w],
                                  in_=sc[:, :w])
                nc.scalar.activation(E[:, v0:v0 + w], ps[:, :w], AF.Exp,
                                     accum_out=lsums[:, j:j + 1])
            lsum = stp.tile([128, 1], F32, tag="lsum")
            nc.vector.tensor_reduce(lsum, lsums, axis=mybir.AxisListType.X,
                                    op=mybir.AluOpType.add)
            lin = dram.tile([128, 1], F32, tag="lin")
            lout = dram.tile([128, 1], F32, tag="lout")
            nc.sync.dma_start(out=lin, in_=lsum)
            nc.gpsimd.collective_compute(
                "AllReduce", mybir.AluOpType.add, replica_groups=RG,
                ins=[lin.opt()], outs=[lout.opt()])
            gs = stp.tile([128, 1], F32, tag="gs")
            nc.sync.dma_start(out=gs, in_=lout)
            nc.vector.reciprocal(scales[:, b:b + 1], gs)
            for j, (v0, w) in enumerate(VT512):
                pr = prp.tile([128, 512], F32)
                nc.vector.tensor_scalar_mul(pr[:, :w], E[:, v0:v0 + w],
                                            scales[:, b:b + 1])
                nc.sync.dma_start(out=probs_d.ap()[r0:r0 + 128, v0:v0 + w],
                                  in_=pr[:, :w])

        # ---- output matmul: kwout_partial = exp(S^T).T @ W_emb ----
        accin = dram_acc.tile([NBLK, 128, D], F32, tag="accin")
        accout = dram_acc.tile([NBLK, 128, D], F32, tag="accout")
        for wv in range(2):          # waves of 4 row-chunks (512 rows)
            r0 = wv * 512
            accs = [ps_acc.tile([128, D], F32, name=f"accps{wv}_{i}",
                                tag=f"acc{i}") for i in range(4)]
            for k, (v0, w) in enumerate(VT128):
                st = ps_st.tile([128, 512], F32)
                for i in range(4):
                    nc.tensor.matmul(st[:w, :],
                                     lhsT=embT[:, i, v0:v0 + w],
                                     rhs=kw_nT[:, i, r0:r0 + 512],
                                     start=(i == 0), stop=(i == 3))
                et = etp.tile([128, 512], BF16)
                nc.scalar.activation(et[:w, :], st[:w, :], AF.Exp)
                for rcl in range(4):
                    nc.tensor.matmul(accs[rcl],
                                     lhsT=et[:w, rcl * 128:(rcl + 1) * 128],
                                     rhs=emb[:w, k, :],
                                     start=(k == 0), stop=(k == len(VT128) - 1))
            for rcl in range(4):
                a = accsb.tile([128, D], F32, tag="acc")
                nc.vector.tensor_copy(a, accs[rcl])
                nc.sync.dma_start(out=accin[wv * 4 + rcl], in_=a)
        nc.gpsimd.collective_compute(
            "AllReduce", mybir.AluOpType.add, replica_groups=RG,
            ins=[accin.opt()], outs=[accout.opt()])
        for rc in range(NBLK):
            ao = accsb.tile([128, D], F32, tag="ao")
            nc.sync.dma_start(out=ao, in_=accout[rc])
            ko = accsb.tile([128, D], F32, tag="ko")
            nc.vector.tensor_scalar_mul(ko, ao, scales[:, rc:rc + 1])
            nc.sync.dma_start(out=kwout_d.ap()[rc * 128:(rc + 1) * 128, :], in_=ko)
    outer.close()


_NC = None
last_results = None


def _get_nc():
    global _NC
    if _NC is None:
        _NC = build_nc()
    return _NC


def kernel(keywords, W_proj, b_proj, W_emb):
    global last_results
    nc = _get_nc()

    kwT = np.ascontiguousarray(
        np.asarray(keywords, dtype=np.float32).reshape(NR, DM).T)
    wproj = np.asarray(W_proj, dtype=np.float32)
    bproj = np.asarray(b_proj, dtype=np.float32).reshape(1, D)
    wbf = np.asarray(W_emb, dtype=np.float32).astype(ml_dtypes.bfloat16)

    in_maps = []
    for c in range(NCORES):
        sh = wbf[c * VSH:(c + 1) * VSH]
        in_maps.append({
            "kwT": kwT,
            "wproj": wproj,
            "bproj": bproj,
            "wemb": np.ascontiguousarray(sh),
            "wembT": np.ascontiguousarray(sh.T),
        })

    res = run_bass_kernel_spmd(nc, in_maps, list(range(NCORES)),
                               trace=bool(os.environ.get("BASS_TRACE")))
    last_results = res
    scores = np.concatenate([res.results[c]["scores"] for c in range(NCORES)],
                            axis=1).reshape(B, KWD, V)
    probs = np.concatenate([res.results[c]["probs"] for c in range(NCORES)],
                           axis=1).reshape(B, KWD, V)
    kwout = np.asarray(res.results[0]["kwout"]).reshape(B, KWD, D)
    return kwout, probs, scores


# revision 10
# speedup vs baseline: 1.0811x; 1.0811x over previous
"""Soft-VQ codebook kernel for Trainium2, 8 NeuronCores, vocab-parallel.

Reference computation (B=128, K=8, d_model=768, d=512, V=49408):
  kw    = keywords @ W_proj + b_proj            # [1024, 512]
  cos   = (kw/|kw|) @ (W_emb/|W_emb|).T         # [1024, V]
  prob  = softmax(cos, axis=-1)
  kwout = prob @ W_emb                          # [1024, 512]
returns (kwout, prob, cos) reshaped to [128, 8, ...].

Sharding: vocab axis of W_emb split 8 ways (6176 rows/core, ~6.3MB bf16
resident in SBUF in both [v,d] and [d,v] layouts). Every core computes all
1024 rows against its vocab shard. Softmax needs no max subtraction
(|cos| <= 1), so the only cross-core exchange is an AllReduce-add of the
per-row exp-sums (one tiny AllReduce per 128-row block) plus one AllReduce
of the [1024, 512] output partial sums.
"""

import os

import numpy as np
import ml_dtypes

import concourse.bass as bass
import concourse.tile as tile
from concourse import bacc, mybir
from concourse.bass_utils import run_bass_kernel_spmd
from concourse.masks import make_identity

F32 = mybir.dt.float32
BF16 = mybir.dt.bfloat16
AF = mybir.ActivationFunctionType

NCORES = 8
B, KWD, DM, D, V = 128, 8, 768, 512, 49408
NR = B * KWD            # 1024 rows
VSH = V // NCORES       # 6176 vocab rows per core
RG = [list(range(NCORES))]

# v-tiles of 512 along the shard vocab axis (12 full + one 32-wide tail)
VT512 = [(j * 512, min(512, VSH - j * 512)) for j in range((VSH + 511) // 512)]
# v-chunks of 128 (48 full + one 32-wide tail)
VT128 = [(k * 128, min(128, VSH - k * 128)) for k in range((VSH + 127) // 128)]
NBLK = NR // 128        # 8 row blocks of 128


def build_nc():
    nc = bacc.Bacc("TRN2", target_bir_lowering=False, debug=False,
                   num_devices=NCORES)

    kwT_d = nc.dram_tensor("kwT", [DM, NR], F32, kind="ExternalInput")
    wproj_d = nc.dram_tensor("wproj", [DM, D], F32, kind="ExternalInput")
    bproj_d = nc.dram_tensor("bproj", [1, D], F32, kind="ExternalInput")
    wembT_d = nc.dram_tensor("wembT", [D, VSH], BF16, kind="ExternalInput")
    wemb_d = nc.dram_tensor("wemb", [VSH, D], BF16, kind="ExternalInput")

    scores_d = nc.dram_tensor("scores", [NR, VSH], F32, kind="ExternalOutput")
    probs_d = nc.dram_tensor("probs", [NR, VSH], F32, kind="ExternalOutput")
    kwout_d = nc.dram_tensor("kwout", [NR, D], F32, kind="ExternalOutput")

    with tile.TileContext(nc) as tc:
        build_tile_kernel(nc, tc, kwT_d, wproj_d, bproj_d, wembT_d, wemb_d,
                          scores_d, probs_d, kwout_d)
    if not nc.is_finalized():
        nc.finalize()
    return nc


def build_tile_kernel(nc, tc, kwT_d, wproj_d, bproj_d, wembT_d, wemb_d,
                      scores_d, probs_d, kwout_d):
    from contextlib import ExitStack

    outer = ExitStack()
    persist = outer.enter_context(tc.tile_pool(name="persist", bufs=1))

    # ---- persistent SBUF ----
    # transposed shard [d, v]: 4 partition-tiles; normalized in place below
    embT = persist.tile([128, 4, VSH], BF16)
    nc.sync.dma_start(out=embT,
                      in_=wembT_d.ap().rearrange("(i p) v -> p i v", p=128))
    # natural shard [v, d]: 49 chunks of <=128 rows (raw W_emb, out-matmul rhs)
    emb = persist.tile([128, len(VT128), D], BF16)
    nfull = (VSH // 128) * 128
    nc.sync.dma_start(out=emb[:, :VSH // 128, :],
                      in_=wemb_d.ap()[:nfull].rearrange("(k p) d -> p k d", p=128))
    tail = VSH - nfull
    if tail:
        nc.sync.dma_start(out=emb[:tail, VSH // 128, :], in_=wemb_d.ap()[nfull:])

    kw_nT = persist.tile([128, 4, NR], BF16)       # normalized kw, transposed
    scales = persist.tile([128, NBLK], F32)        # 1/gsum per row block
    ident = persist.tile([128, 128], BF16)
    make_identity(nc, ident)
    ones_bf = persist.tile([128, 1], BF16)
    nc.vector.memset(ones_bf, 1.0)
    ones_row = persist.tile([1, 128], F32)
    nc.vector.memset(ones_row, 1.0)

    # ---- setup: normalize embT in place;  compute kw_nT ----
    with ExitStack() as setup:
        sp = setup.enter_context(tc.tile_pool(name="setup", bufs=2))
        kwp = setup.enter_context(tc.tile_pool(name="kwsetup", bufs=1))
        ps_n = setup.enter_context(tc.tile_pool(name="psn", bufs=2, space="PSUM"))
        ps_b = setup.enter_context(tc.tile_pool(name="psb", bufs=2, space="PSUM"))
        ps_kw = setup.enter_context(tc.tile_pool(name="pskw", bufs=2, space="PSUM"))
        ps_t = setup.enter_context(tc.tile_pool(name="pst", bufs=2, space="PSUM"))

        # vocab norms: norm2[v] = sum_d W[v,d]^2 via ones-matmul over 4 d-tiles
        for v0, w in VT512:
            n2 = ps_n.tile([1, 512], F32)
            for i in range(4):
                sq = sp.tile([128, 512], BF16, tag="sq")
                nc.vector.tensor_mul(sq[:, :w], embT[:, i, v0:v0 + w],
                                     embT[:, i, v0:v0 + w])
                nc.tensor.matmul(n2[:, :w], lhsT=ones_bf, rhs=sq[:, :w],
                                 start=(i == 0), stop=(i == 3))
            nrm = sp.tile([1, 512], F32, tag="nrm")
            nc.scalar.sqrt(nrm[:, :w], n2[:, :w])
            inv = sp.tile([1, 512], F32, tag="inv")
            nc.vector.reciprocal(inv[:, :w], nrm[:, :w])
            bc = ps_b.tile([128, 512], F32)
            nc.tensor.matmul(bc[:, :w], lhsT=ones_row, rhs=inv[:, :w],
                             start=True, stop=True)
            for i in range(4):
                nc.vector.tensor_mul(embT[:, i, v0:v0 + w],
                                     embT[:, i, v0:v0 + w], bc[:, :w])

        # kw = keywords @ W_proj + b;  kw_n = kw/|kw|;  kw_nT = transpose
        kwT_sb = kwp.tile([128, 6, NR], F32)
        nc.sync.dma_start(out=kwT_sb,
                          in_=kwT_d.ap().rearrange("(i p) r -> p i r", p=128))
        wproj_sb = kwp.tile([128, 6, D], F32)
        nc.sync.dma_start(out=wproj_sb,
                          in_=wproj_d.ap().rearrange("(i p) d -> p i d", p=128))
        bproj_sb = kwp.tile([1, D], F32)
        nc.sync.dma_start(out=bproj_sb, in_=bproj_d.ap())

        for rc in range(NBLK):
            kw_ps = ps_kw.tile([128, D], F32)
            for i in range(6):
                nc.tensor.matmul(kw_ps, lhsT=kwT_sb[:, i, rc * 128:(rc + 1) * 128],
                                 rhs=wproj_sb[:, i, :], start=(i == 0), stop=False)
            # bias add: += ones[128] (x) b_proj  (K=1 matmul)
            nc.tensor.matmul(kw_ps, lhsT=ones_row, rhs=bproj_sb,
                             start=False, stop=True)
            sqk = sp.tile([128, D], F32, tag="sqk")
            n2k = sp.tile([128, 1], F32, tag="n2k")
            nc.scalar.activation(sqk, kw_ps, AF.Square, accum_out=n2k)
            nrk = sp.tile([128, 1], F32, tag="nrk")
            nc.scalar.sqrt(nrk, n2k)
            ivk = sp.tile([128, 1], F32, tag="ivk")
            nc.vector.reciprocal(ivk, nrk)
            kwn = sp.tile([128, D], BF16, tag="kwn")
            nc.vector.tensor_scalar_mul(kwn, kw_ps, ivk)
            for i in range(4):
                tp = ps_t.tile([128, 128], BF16)
                nc.tensor.transpose(tp, kwn[:, i * 128:(i + 1) * 128], ident)
                nc.scalar.copy(kw_nT[:, i, rc * 128:(rc + 1) * 128], tp)

    # ---- main pools ----
    with ExitStack() as main:
        ep = main.enter_context(tc.tile_pool(name="efwd", bufs=4))
        scp = main.enter_context(tc.tile_pool(name="scstage", bufs=4))
        prp = main.enter_context(tc.tile_pool(name="prstage", bufs=4))
        lsp = main.enter_context(tc.tile_pool(name="lsums", bufs=2))
        stp = main.enter_context(tc.tile_pool(name="stats", bufs=4))
        etp = main.enter_context(tc.tile_pool(name="et", bufs=3))
        accsb = main.enter_context(tc.tile_pool(name="accsb", bufs=4))
        ps_s = main.enter_context(tc.tile_pool(name="pss", bufs=2, space="PSUM"))
        ps_st = main.enter_context(tc.tile_pool(name="psst", bufs=2, space="PSUM"))
        ps_acc = main.enter_context(tc.tile_pool(name="psacc", bufs=1, space="PSUM"))
        dram = main.enter_context(tc.tile_pool(name="dram", bufs=4, space="DRAM"))
        dram_acc = main.enter_context(tc.tile_pool(name="dramacc", bufs=2,
                                                   space="DRAM"))

        # ---- fused schedule ----
        # S-phase per 128-row block (scores, exp, row-sums) interleaved with
        # slices of the output matmul (exp(S^T).T @ W_emb) so TensorE stays
        # dense while the softmax-sum AllReduces are in flight. The lsum
        # AllReduce is batched over pairs of blocks (4 collectives, not 8).
        accin = dram_acc.tile([NBLK, 128, D], F32, tag="accin")
        accout = dram_acc.tile([NBLK, 128, D], F32, tag="accout")
        NK = len(VT128)
        # split the 49 v-chunks of each wave across its 4 host blocks
        ksplit = [0, 13, 25, 37, NK]
        accs = None
        lsum2 = None

        def wave_slice(b):
            nonlocal accs
            wv, sl = divmod(b, 4)
            r0 = wv * 512
            if sl == 0:
                accs = [ps_acc.tile([128, D], F32, name=f"accps{wv}_{i}",
                                    tag=f"acc{i}") for i in range(4)]
            for k in range(ksplit[sl], ksplit[sl + 1]):
                v0, w = VT128[k]
                st = ps_st.tile([128, 512], F32, name="st")
                for i in range(4):
                    nc.tensor.matmul(st[:w, :],
                                     lhsT=embT[:, i, v0:v0 + w],
                                     rhs=kw_nT[:, i, r0:r0 + 512],
                                     start=(i == 0), stop=(i == 3))
                et = etp.tile([128, 512], BF16, name="et")
                nc.scalar.activation(et[:w, :], st[:w, :], AF.Exp)
                for rcl in range(4):
                    nc.tensor.matmul(accs[rcl],
                                     lhsT=et[:w, rcl * 128:(rcl + 1) * 128],
                                     rhs=emb[:w, k, :],
                                     start=(k == 0), stop=(k == NK - 1))
            if sl == 3:
                for rcl in range(4):
                    a = accsb.tile([128, D], F32, tag="acc")
                    nc.vector.tensor_copy(a, accs[rcl])
                    nc.sync.dma_start(out=accin[wv * 4 + rcl], in_=a)

        Etiles = {}
        for b in range(NBLK):
            r0 = b * 128
            par = b % 2
            E = ep.tile([128, VSH], BF16)
            Etiles[b] = E
            lsums = lsp.tile([128, len(VT512)], F32)
            if par == 0:
                lsum2 = stp.tile([128, 2], F32, tag="lsum2")
            for j, (v0, w) in enumerate(VT512):
                ps = ps_s.tile([128, 512], F32)
                for i in range(4):
                    nc.tensor.matmul(ps[:, :w],
                                     lhsT=kw_nT[:, i, r0:r0 + 128],
                                     rhs=embT[:, i, v0:v0 + w],
                                     start=(i == 0), stop=(i == 3))
                sc = scp.tile([128, 512], F32)
                nc.vector.tensor_copy(sc[:, :w], ps[:, :w])
                nc.sync.dma_start(out=scores_d.ap()[r0:r0 + 128, v0:v0 + w],
                                  in_=sc[:, :w])
                nc.scalar.activation(E[:, v0:v0 + w], ps[:, :w], AF.Exp,
                                     accum_out=lsums[:, j:j + 1])
            nc.vector.tensor_reduce(lsum2[:, par:par + 1], lsums,
                                    axis=mybir.AxisListType.X,
                                    op=mybir.AluOpType.add)
            wave_slice(b)
            if par == 1:
                lin = dram.tile([128, 2], F32, tag="lin")
                lout = dram.tile([128, 2], F32, tag="lout")
                nc.sync.dma_start(out=lin, in_=lsum2)
                nc.gpsimd.collective_compute(
                    "AllReduce", mybir.AluOpType.add, replica_groups=RG,
                    ins=[lin.opt()], outs=[lout.opt()])
                gs = stp.tile([128, 2], F32, tag="gs")
                nc.sync.dma_start(out=gs, in_=lout)
                nc.vector.reciprocal(scales[:, b - 1:b + 1], gs)
                for bb in (b - 1, b):
                    Eb = Etiles.pop(bb)
                    rb = bb * 128
                    for j, (v0, w) in enumerate(VT512):
                        pr = prp.tile([128, 512], F32)
                        nc.vector.tensor_scalar_mul(
                            pr[:, :w], Eb[:, v0:v0 + w],
                            scales[:, bb:bb + 1])
                        nc.sync.dma_start(
                            out=probs_d.ap()[rb:rb + 128, v0:v0 + w],
                            in_=pr[:, :w])
        nc.gpsimd.collective_compute(
            "AllReduce", mybir.AluOpType.add, replica_groups=RG,
            ins=[accin.opt()], outs=[accout.opt()])
        for rc in range(NBLK):
            ao = accsb.tile([128, D], F32, tag="ao")
            nc.sync.dma_start(out=ao, in_=accout[rc])
            ko = accsb.tile([128, D], F32, tag="ko")
            nc.vector.tensor_scalar_mul(ko, ao, scales[:, rc:rc + 1])
            nc.sync.dma_start(out=kwout_d.ap()[rc * 128:(rc + 1) * 128, :], in_=ko)
    outer.close()


_NC = None
last_results = None


def _get_nc():
    global _NC
    if _NC is None:
        _NC = build_nc()
    return _NC


def kernel(keywords, W_proj, b_proj, W_emb):
    global last_results
    nc = _get_nc()

    kwT = np.ascontiguousarray(
        np.asarray(keywords, dtype=np.float32).reshape(NR, DM).T)
    wproj = np.asarray(W_proj, dtype=np.float32)
    bproj = np.asarray(b_proj, dtype=np.float32).reshape(1, D)
    wbf = np.asarray(W_emb, dtype=np.float32).astype(ml_dtypes.bfloat16)

    in_maps = []
    for c in range(NCORES):
        sh = wbf[c * VSH:(c + 1) * VSH]
        in_maps.append({
            "kwT": kwT,
            "wproj": wproj,
            "bproj": bproj,
            "wemb": np.ascontiguousarray(sh),
            "wembT": np.ascontiguousarray(sh.T),
        })

    res = run_bass_kernel_spmd(nc, in_maps, list(range(NCORES)),
                               trace=bool(os.environ.get("BASS_TRACE")))
    last_results = res
    scores = np.concatenate([res.results[c]["scores"] for c in range(NCORES)],
                            axis=1).reshape(B, KWD, V)
    probs = np.concatenate([res.results[c]["probs"] for c in range(NCORES)],
                           axis=1).reshape(B, KWD, V)
    kwout = np.asarray(res.results[0]["kwout"]).reshape(B, KWD, D)
    return kwout, probs, scores


# revision 14
# speedup vs baseline: 1.1240x; 1.0397x over previous
"""Soft-VQ codebook kernel for Trainium2, 8 NeuronCores, vocab-parallel.

Reference computation (B=128, K=8, d_model=768, d=512, V=49408):
  kw    = keywords @ W_proj + b_proj            # [1024, 512]
  cos   = (kw/|kw|) @ (W_emb/|W_emb|).T         # [1024, V]
  prob  = softmax(cos, axis=-1)
  kwout = prob @ W_emb                          # [1024, 512]
returns (kwout, prob, cos) reshaped to [128, 8, ...].

Sharding: vocab axis of W_emb split 8 ways (6176 rows/core, ~6.3MB bf16
resident in SBUF in both [v,d] and [d,v] layouts). Every core computes all
1024 rows against its vocab shard. Softmax needs no max subtraction
(|cos| <= 1), so the only cross-core exchange is an AllReduce-add of the
per-row exp-sums (one tiny AllReduce per 128-row block) plus one AllReduce
of the [1024, 512] output partial sums.
"""

import os

import numpy as np
import ml_dtypes

import concourse.bass as bass
import concourse.tile as tile
from concourse import bacc, mybir
from concourse.bass_utils import run_bass_kernel_spmd
from concourse.masks import make_identity

F32 = mybir.dt.float32
BF16 = mybir.dt.bfloat16
AF = mybir.ActivationFunctionType

NCORES = 8
B, KWD, DM, D, V = 128, 8, 768, 512, 49408
NR = B * KWD            # 1024 rows
VSH = V // NCORES       # 6176 vocab rows per core
RG = [list(range(NCORES))]

# v-tiles of 512 along the shard vocab axis (12 full + one 32-wide tail)
VT512 = [(j * 512, min(512, VSH - j * 512)) for j in range((VSH + 511) // 512)]
# v-chunks of 128 (48 full + one 32-wide tail)
VT128 = [(k * 128, min(128, VSH - k * 128)) for k in range((VSH + 127) // 128)]
NBLK = NR // 128        # 8 row blocks of 128


def build_nc():
    nc = bacc.Bacc("TRN2", target_bir_lowering=False, debug=False,
                   num_devices=NCORES)

    kwT_d = nc.dram_tensor("kwT", [DM, NR], F32, kind="ExternalInput")
    wproj_d = nc.dram_tensor("wproj", [DM, D], F32, kind="ExternalInput")
    bproj_d = nc.dram_tensor("bproj", [1, D], F32, kind="ExternalInput")
    wembT_d = nc.dram_tensor("wembT", [D, VSH], BF16, kind="ExternalInput")
    wemb_d = nc.dram_tensor("wemb", [VSH, D], BF16, kind="ExternalInput")

    scores_d = nc.dram_tensor("scores", [NR, VSH], F32, kind="ExternalOutput")
    probs_d = nc.dram_tensor("probs", [NR, VSH], F32, kind="ExternalOutput")
    kwout_d = nc.dram_tensor("kwout", [NR, D], F32, kind="ExternalOutput")

    with tile.TileContext(nc) as tc:
        build_tile_kernel(nc, tc, kwT_d, wproj_d, bproj_d, wembT_d, wemb_d,
                          scores_d, probs_d, kwout_d)
    if not nc.is_finalized():
        nc.finalize()
    return nc


def build_tile_kernel(nc, tc, kwT_d, wproj_d, bproj_d, wembT_d, wemb_d,
                      scores_d, probs_d, kwout_d):
    from contextlib import ExitStack

    outer = ExitStack()
    persist = outer.enter_context(tc.tile_pool(name="persist", bufs=1))

    # ---- persistent SBUF ----
    # transposed shard [d, v]: 4 partition-tiles; normalized in place below
    embT = persist.tile([128, 4, VSH], BF16)
    nc.sync.dma_start(out=embT,
                      in_=wembT_d.ap().rearrange("(i p) v -> p i v", p=128))
    # natural shard [v, d]: 49 chunks of <=128 rows (raw W_emb, out-matmul rhs)
    emb = persist.tile([128, len(VT128), D], BF16)
    nfull = (VSH // 128) * 128
    nc.sync.dma_start(out=emb[:, :VSH // 128, :],
                      in_=wemb_d.ap()[:nfull].rearrange("(k p) d -> p k d", p=128))
    tail = VSH - nfull
    if tail:
        nc.sync.dma_start(out=emb[:tail, VSH // 128, :], in_=wemb_d.ap()[nfull:])

    kw_nT = persist.tile([128, 4, NR], BF16)       # normalized kw, transposed
    scales = persist.tile([128, NBLK], F32)        # 1/gsum per row block
    ident = persist.tile([128, 128], BF16)
    make_identity(nc, ident)
    ones_bf = persist.tile([128, 1], BF16)
    nc.vector.memset(ones_bf, 1.0)
    ones_row = persist.tile([1, 128], F32)
    nc.vector.memset(ones_row, 1.0)

    # ---- setup: normalize embT in place;  compute kw_nT ----
    with ExitStack() as setup:
        sp = setup.enter_context(tc.tile_pool(name="setup", bufs=2))
        kwp = setup.enter_context(tc.tile_pool(name="kwsetup", bufs=1))
        ps_n = setup.enter_context(tc.tile_pool(name="psn", bufs=2, space="PSUM"))
        ps_b = setup.enter_context(tc.tile_pool(name="psb", bufs=2, space="PSUM"))
        ps_kw = setup.enter_context(tc.tile_pool(name="pskw", bufs=2, space="PSUM"))
        ps_t = setup.enter_context(tc.tile_pool(name="pst", bufs=2, space="PSUM"))

        # kw = keywords @ W_proj + b;  kw_n = kw/|kw|;  kw_nT = transpose
        # (emitted before the vocab-norm phase so PE has work while the
        # 12.6MB embT DMA is still landing)
        kwT_sb = kwp.tile([128, 6, NR], F32)
        nc.sync.dma_start(out=kwT_sb,
                          in_=kwT_d.ap().rearrange("(i p) r -> p i r", p=128))
        wproj_sb = kwp.tile([128, 6, D], F32)
        nc.sync.dma_start(out=wproj_sb,
                          in_=wproj_d.ap().rearrange("(i p) d -> p i d", p=128))
        bproj_sb = kwp.tile([1, D], F32)
        nc.sync.dma_start(out=bproj_sb, in_=bproj_d.ap())

        for rc in range(NBLK):
            kw_ps = ps_kw.tile([128, D], F32)
            for i in range(6):
                nc.tensor.matmul(kw_ps, lhsT=kwT_sb[:, i, rc * 128:(rc + 1) * 128],
                                 rhs=wproj_sb[:, i, :], start=(i == 0), stop=False)
            # bias add: += ones[128] (x) b_proj  (K=1 matmul)
            nc.tensor.matmul(kw_ps, lhsT=ones_row, rhs=bproj_sb,
                             start=False, stop=True)
            sqk = sp.tile([128, D], F32, tag="sqk")
            n2k = sp.tile([128, 1], F32, tag="n2k")
            nc.scalar.activation(sqk, kw_ps, AF.Square, accum_out=n2k)
            nrk = sp.tile([128, 1], F32, tag="nrk")
            nc.scalar.sqrt(nrk, n2k)
            ivk = sp.tile([128, 1], F32, tag="ivk")
            nc.vector.reciprocal(ivk, nrk)
            kwn = sp.tile([128, D], BF16, tag="kwn")
            nc.vector.tensor_scalar_mul(kwn, kw_ps, ivk)
            for i in range(4):
                tp = ps_t.tile([128, 128], BF16)
                nc.tensor.transpose(tp, kwn[:, i * 128:(i + 1) * 128], ident)
                nc.scalar.copy(kw_nT[:, i, rc * 128:(rc + 1) * 128], tp)

        # vocab norms: norm2[v] = sum_d W[v,d]^2 via ones-matmul over 4 d-tiles
        for v0, w in VT512:
            n2 = ps_n.tile([1, 512], F32)
            for i in range(4):
                sq = sp.tile([128, 512], BF16, tag="sq")
                nc.vector.tensor_mul(sq[:, :w], embT[:, i, v0:v0 + w],
                                     embT[:, i, v0:v0 + w])
                nc.tensor.matmul(n2[:, :w], lhsT=ones_bf, rhs=sq[:, :w],
                                 start=(i == 0), stop=(i == 3))
            nrm = sp.tile([1, 512], F32, tag="nrm")
            nc.scalar.sqrt(nrm[:, :w], n2[:, :w])
            inv = sp.tile([1, 512], F32, tag="inv")
            nc.vector.reciprocal(inv[:, :w], nrm[:, :w])
            bc = ps_b.tile([128, 512], F32)
            nc.tensor.matmul(bc[:, :w], lhsT=ones_row, rhs=inv[:, :w],
                             start=True, stop=True)
            for i in range(4):
                nc.vector.tensor_mul(embT[:, i, v0:v0 + w],
                                     embT[:, i, v0:v0 + w], bc[:, :w])

    # ---- main pools ----
    with ExitStack() as main:
        ep = main.enter_context(tc.tile_pool(name="efwd", bufs=4))
        scp = main.enter_context(tc.tile_pool(name="scstage", bufs=4))
        prp = main.enter_context(tc.tile_pool(name="prstage", bufs=4))
        lsp = main.enter_context(tc.tile_pool(name="lsums", bufs=2))
        stp = main.enter_context(tc.tile_pool(name="stats", bufs=4))
        etp = main.enter_context(tc.tile_pool(name="et", bufs=3))
        accsb = main.enter_context(tc.tile_pool(name="accsb", bufs=4))
        ps_s = main.enter_context(tc.tile_pool(name="pss", bufs=2, space="PSUM"))
        ps_st = main.enter_context(tc.tile_pool(name="psst", bufs=2, space="PSUM"))
        ps_acc = main.enter_context(tc.tile_pool(name="psacc", bufs=1, space="PSUM"))
        dram = main.enter_context(tc.tile_pool(name="dram", bufs=4, space="DRAM"))
        dram_acc = main.enter_context(tc.tile_pool(name="dramacc", bufs=2,
                                                   space="DRAM"))

        # ---- S phases (all 8 blocks): scores, exp, row sums; the lsum
        # AllReduce is batched over pairs of blocks (4 collectives). ----
        NK = len(VT128)
        lsum2 = None
        Etiles = {}
        for b in range(NBLK):
            r0 = b * 128
            par = b % 2
            E = ep.tile([128, VSH], BF16)
            Etiles[b] = E
            lsums = lsp.tile([128, len(VT512)], F32)
            if par == 0:
                lsum2 = stp.tile([128, 2], F32, tag="lsum2")
            for j, (v0, w) in enumerate(VT512):
                ps = ps_s.tile([128, 512], F32)
                for i in range(4):
                    nc.tensor.matmul(ps[:, :w],
                                     lhsT=kw_nT[:, i, r0:r0 + 128],
                                     rhs=embT[:, i, v0:v0 + w],
                                     start=(i == 0), stop=(i == 3))
                sc = scp.tile([128, 512], F32)
                nc.vector.tensor_copy(sc[:, :w], ps[:, :w])
                nc.sync.dma_start(out=scores_d.ap()[r0:r0 + 128, v0:v0 + w],
                                  in_=sc[:, :w])
                nc.scalar.activation(E[:, v0:v0 + w], ps[:, :w], AF.Exp,
                                     accum_out=lsums[:, j:j + 1])
            nc.vector.tensor_reduce(lsum2[:, par:par + 1], lsums,
                                    axis=mybir.AxisListType.X,
                                    op=mybir.AluOpType.add)
            if par == 1:
                lin = dram.tile([128, 2], F32, tag="lin")
                lout = dram.tile([128, 2], F32, tag="lout")
                nc.sync.dma_start(out=lin, in_=lsum2)
                nc.gpsimd.collective_compute(
                    "AllReduce", mybir.AluOpType.add, replica_groups=RG,
                    ins=[lin.opt()], outs=[lout.opt()])
                gs = stp.tile([128, 2], F32, tag="gs")
                nc.sync.dma_start(out=gs, in_=lout)
                nc.vector.reciprocal(scales[:, b - 1:b + 1], gs)
                for bb in (b - 1, b):
                    Eb = Etiles.pop(bb)
                    rb = bb * 128
                    for j, (v0, w) in enumerate(VT512):
                        pr = prp.tile([128, 512], F32)
                        nc.vector.tensor_scalar_mul(
                            pr[:, :w], Eb[:, v0:v0 + w],
                            scales[:, bb:bb + 1])
                        nc.sync.dma_start(
                            out=probs_d.ap()[rb:rb + 128, v0:v0 + w],
                            in_=pr[:, :w])

        # ---- output matmul: kwout_partial = exp(S^T).T @ W_emb; runs after
        # the S phases so its dense PE work overlaps the probs/AllReduce
        # drain. The partial-sum AllReduce is split per 512-row wave so the
        # first one overlaps the second wave's compute. ----
        for wv in range(2):          # waves of 4 row-chunks (512 rows)
            r0 = wv * 512
            accin = dram_acc.tile([4, 128, D], F32, tag="accin")
            accout = dram_acc.tile([4, 128, D], F32, tag="accout")
            accs = [ps_acc.tile([128, D], F32, name=f"accps{wv}_{i}",
                                tag=f"acc{i}") for i in range(4)]
            for k, (v0, w) in enumerate(VT128):
                st = ps_st.tile([128, 512], F32, name="st")
                for i in range(4):
                    nc.tensor.matmul(st[:w, :],
                                     lhsT=embT[:, i, v0:v0 + w],
                                     rhs=kw_nT[:, i, r0:r0 + 512],
                                     start=(i == 0), stop=(i == 3))
                et = etp.tile([128, 512], BF16, name="et")
                nc.scalar.activation(et[:w, :], st[:w, :], AF.Exp)
                for rcl in range(4):
                    nc.tensor.matmul(accs[rcl],
                                     lhsT=et[:w, rcl * 128:(rcl + 1) * 128],
                                     rhs=emb[:w, k, :],
                                     start=(k == 0), stop=(k == NK - 1))
            for rcl in range(4):
                a = accsb.tile([128, D], F32, tag="acc")
                nc.vector.tensor_copy(a, accs[rcl])
                nc.sync.dma_start(out=accin[rcl], in_=a)
            nc.gpsimd.collective_compute(
                "AllReduce", mybir.AluOpType.add, replica_groups=RG,
                ins=[accin.opt()], outs=[accout.opt()])
            for rcl in range(4):
                rc = wv * 4 + rcl
                ao = accsb.tile([128, D], F32, tag="ao")
                nc.sync.dma_start(out=ao, in_=accout[rcl])
                ko = accsb.tile([128, D], F32, tag="ko")
                nc.vector.tensor_scalar_mul(ko, ao, scales[:, rc:rc + 1])
                nc.sync.dma_start(out=kwout_d.ap()[rc * 128:(rc + 1) * 128, :],
                                  in_=ko)
    outer.close()


_NC = None
last_results = None


def _get_nc():
    global _NC
    if _NC is None:
        _NC = build_nc()
    return _NC


def kernel(keywords, W_proj, b_proj, W_emb):
    global last_results
    nc = _get_nc()

    kwT = np.ascontiguousarray(
        np.asarray(keywords, dtype=np.float32).reshape(NR, DM).T)
    wproj = np.asarray(W_proj, dtype=np.float32)
    bproj = np.asarray(b_proj, dtype=np.float32).reshape(1, D)
    wbf = np.asarray(W_emb, dtype=np.float32).astype(ml_dtypes.bfloat16)

    in_maps = []
    for c in range(NCORES):
        sh = wbf[c * VSH:(c + 1) * VSH]
        in_maps.append({
            "kwT": kwT,
            "wproj": wproj,
            "bproj": bproj,
            "wemb": np.ascontiguousarray(sh),
            "wembT": np.ascontiguousarray(sh.T),
        })

    res = run_bass_kernel_spmd(nc, in_maps, list(range(NCORES)),
                               trace=bool(os.environ.get("BASS_TRACE")))
    last_results = res
    scores = np.concatenate([res.results[c]["scores"] for c in range(NCORES)],
                            axis=1).reshape(B, KWD, V)
    probs = np.concatenate([res.results[c]["probs"] for c in range(NCORES)],
                           axis=1).reshape(B, KWD, V)
    kwout = np.asarray(res.results[0]["kwout"]).reshape(B, KWD, D)
    return kwout, probs, scores
